# revision 2
# baseline (speedup 1.0000x reference)
"""Trainium2 Bass kernel for nn_LongAttention (holographic long-attention block).

Computation (see reference):
  raw = x @ W_in.T -> split [c_phase | c_mag | q_re | q_im] per hd channel
  key = sigmoid(c_mag) * exp(i*(pi*tanh(c_phase) + pos_phase))
  state = cumsum_t(key);  ret = state * conj(q)
  ret_real = interleave(Re, Im) -> LayerNorm(2*hd) -> @ W_out.T

Distribution: hd (8192) split across 8 NeuronCores (1024 ch each); every core
handles both batches and all tokens. Cores are fully independent:
 - gamma is folded into W_out on the host; LayerNorm itself is algebraically
   deferred: each core returns P = ret @ (W_out*gamma).T partials plus the raw
   retrieval tiles (f16). The host computes per-token S1/S2 from the retrieval
   dump and combines:
   out = istd * (sum_c P_c - mu * (W_out @ gamma)) + W_out @ beta.
 - The cumsum runs channel-major on the DVE as a prefix scan along the free
   (time) axis, carried across token chunks -- no transposes anywhere.
 - sin/cos are evaluated via the angle-addition formula with host-precomputed
   0.5*cos/0.5*sin of pos_phases (fp16; the 0.5 cancels the sigmoid's
   (tanh+1)/2), so every ACT Sin argument is in [-pi, pi] by construction
   (the hardware LUT's valid range).

Matmuls run in fp16 (more mantissa than bf16 at the same 1 cycle/row rate),
accumulating in fp32. The elementwise chain is fp16 end-to-end: every DVE
tensor_tensor op with all-16-bit packed SBUF operands gets the 2x perf mode,
halving vector-engine busy time vs fp32.
"""

import sys
import numpy as np

for _p in ("/opt/trn_rl_repo", "/root/.axon_site/_ro/trn_rl_repo"):
    if _p not in sys.path:
        sys.path.append(_p)

import bass_rust
import concourse.bass as bass
import concourse.tile as tile
import concourse.mybir as mybir
from concourse.bass_utils import run_bass_kernel_spmd

F32 = mybir.dt.float32
F16 = mybir.dt.float16
BF16 = mybir.dt.bfloat16
AF = mybir.ActivationFunctionType
ALU = mybir.AluOpType
PI = float(np.pi)

N_CORES = 8
LN_EPS = 1e-5


# --------------------------------------------------------------------------
# Workaround: this container's walrus rejects >1 semaphore wait per
# instruction ("Too many sync wait commands"). Split the extras onto
# same-engine NoOps inserted just before (engine FIFO keeps semantics).
# --------------------------------------------------------------------------
_nop_counter = [0]


def split_multiwait(nc):
    n_split = 0
    for f in nc.m.functions:
        for bb in f.blocks:
            il = bb.instructions
            i = 0
            while i < len(il):
                ins = il[i]
                si = ins.sync_info
                waits = list(si.on_wait) if si is not None and si.on_wait else []
                if len(waits) > 1:
                    for w in waits[:-1]:
                        _nop_counter[0] += 1
                        nop = bass_rust.InstNoOp(
                            name=f"mw_nop_{_nop_counter[0]}",
                            engine=ins.engine,
                            ins=[],
                            outs=[],
                        )
                        nop.sync_info = mybir.SyncInfo(on_wait=[w], on_update=[])
                        il.insert(i, nop)
                        i += 1
                    si.on_wait = [waits[-1]]
                    n_split += 1
                i += 1
    return n_split


# --------------------------------------------------------------------------
# Device program (SPMD: identical on all cores; per-core data differs)
# --------------------------------------------------------------------------
class Cfg:
    def __init__(self, B=2, T=2048, DIM=1024, NCH=1024, CN=256):
        self.B, self.T, self.DIM, self.NCH, self.CN = B, T, DIM, NCH, CN
        self.NTOK = B * T
        self.CT = NCH // 128          # channel tiles per core
        self.KT1 = DIM // 128         # contraction tiles for proj_in
        self.KT2 = 2 * self.CT        # contraction tiles for proj_out (re+im)
        self.DT = DIM // 128          # output dim tiles
        self.NCHUNK = self.NTOK // CN
        self.CPB = T // CN            # chunks per batch


def build_program(cfg: Cfg, reps: int = 1):
    c = cfg
    assert c.CT % 4 == 0 or c.CT == 2
    SEGS = 4 if c.CT % 4 == 0 else 2   # channel tiles per wide tile
    NH = c.CT // SEGS                  # wide halves per chunk
    W = SEGS * c.CN                    # wide tile width
    nc = bass.Bass()

    w1 = nc.dram_tensor("w1", [128, c.KT1, 4 * c.NCH], F16, kind="ExternalInput")
    w2 = nc.dram_tensor("w2", [128, c.KT2, c.DIM], F16, kind="ExternalInput")
    xt = nc.dram_tensor("xt", [128, c.KT1, c.NTOK], F16, kind="ExternalInput")
    cp = nc.dram_tensor("cp", [128, c.CT, c.T], F16, kind="ExternalInput")
    sp = nc.dram_tensor("sp", [128, c.CT, c.T], F16, kind="ExternalInput")
    outp = nc.dram_tensor("outp", [128, c.DT, c.NTOK], F16, kind="ExternalOutput")
    retd = nc.dram_tensor("retd", [128, c.KT2, c.NTOK], F16, kind="ExternalOutput")

    from contextlib import ExitStack
    with tile.TileContext(nc) as tc, ExitStack() as es:
        consts = es.enter_context(tc.tile_pool(name="consts", bufs=1))
        stream = es.enter_context(tc.tile_pool(name="stream", bufs=2))
        wide = es.enter_context(tc.tile_pool(name="wide", bufs=2))
        retp = es.enter_context(tc.tile_pool(name="retp", bufs=2))
        obp = es.enter_context(tc.tile_pool(name="obp", bufs=2))
        praw = es.enter_context(tc.tile_pool(name="praw", bufs=4, space="PSUM"))
        pout = es.enter_context(tc.tile_pool(name="pout", bufs=2, space="PSUM"))

        # weights: split the load per k-tile so the first chunk's matmuls can
        # start as soon as their slice lands (instead of after the full 12 MB)
        w1_sb = consts.tile([128, c.KT1, 4 * c.NCH], F16, tag="w1_sb")
        for k in range(c.KT1):
            nc.sync.dma_start(out=w1_sb[:, k, :], in_=w1[:, k, :])
        w2_sb = consts.tile([128, c.KT2, c.DIM], F16, tag="w2_sb")
        for k in range(c.KT2):
            nc.sync.dma_start(out=w2_sb[:, k, :], in_=w2[:, k, :])
        w1_t = [w1_sb[:, k, :] for k in range(c.KT1)]
        w2_t = [w2_sb[:, k, :] for k in range(c.KT2)]

        half_pi = consts.tile([128, 1], F32, tag="half_pi")
        nc.vector.memset(half_pi[:], PI / 2)
        car = {}
        for h in range(NH):
            for pl in ("re", "im"):
                car[(h, pl)] = consts.tile([128, SEGS], F32, tag=f"car_{h}_{pl}",
                                           name=f"car_{h}_{pl}")

        for n in [nn_ for _ in range(reps) for nn_ in range(c.NCHUNK)]:
            t0 = (n % c.CPB) * c.CN
            first_in_batch = t0 == 0
            tok = slice(n * c.CN, (n + 1) * c.CN)

            xcb = stream.tile([128, c.KT1, c.CN], F16, tag="xcb")
            nc.sync.dma_start(out=xcb[:], in_=xt[:, :, tok])
            xc = [xcb[:, k, :] for k in range(c.KT1)]
            cpb = stream.tile([128, c.CT, c.CN], F16, tag="cpb")
            nc.sync.dma_start(out=cpb[:], in_=cp[:, :, t0:t0 + c.CN])
            spb = stream.tile([128, c.CT, c.CN], F16, tag="spb")
            nc.sync.dma_start(out=spb[:], in_=sp[:, :, t0:t0 + c.CN])

            ret_w = {}
            for h in range(NH):
                i0 = h * SEGS
                # ---- proj_in: 4 groups x SEGS channel tiles -> psum pairs ----
                # psum tile [128, 2*CN] holds channel tiles (j, j+1) of a group
                th_ph = wide.tile([128, W], F16, tag="th_ph", name="th_ph")
                th_mg = wide.tile([128, W], F16, tag="th_mg", name="th_mg")
                qre = wide.tile([128, W], F16, tag="qre", name="qre")
                qim = wide.tile([128, W], F16, tag="qim", name="qim")
                dest = {"ph": th_ph, "mg": th_mg, "qr": qre, "qi": qim}
                for j in range(0, SEGS, 2):
                    for gi, g in enumerate(("ph", "mg", "qr", "qi")):
                        p = praw.tile([128, 2 * c.CN], F32, tag="praw")
                        for half in range(2):
                            m = gi * c.CT + i0 + j + half
                            cols = slice(half * c.CN, (half + 1) * c.CN)
                            for k in range(c.KT1):
                                nc.tensor.matmul(
                                    p[:, cols],
                                    w1_t[k][:, m * 128:(m + 1) * 128], xc[k],
                                    start=(k == 0), stop=(k == c.KT1 - 1))
                        wcols = slice(j * c.CN, (j + 2) * c.CN)
                        if g == "ph" or g == "mg":
                            sc = 1.0 if g == "ph" else 0.5
                            nc.scalar.activation(dest[g][:, wcols], p[:],
                                                 AF.Tanh, scale=sc)
                        else:
                            nc.scalar.copy(dest[g][:, wcols], p[:])

                # ---- content phasor (wide) ----
                sinp = wide.tile([128, W], F16, tag="sinp", name="sinp")
                nc.scalar.activation(sinp[:], th_ph[:], AF.Sin, scale=PI)
                tabs = wide.tile([128, W], F16, tag="tabs", name="tabs")
                nc.scalar.activation(tabs[:], th_ph[:], AF.Abs)
                cosp = wide.tile([128, W], F16, tag="th_ph", name="cosp")
                nc.scalar.activation(cosp[:], tabs[:], AF.Sin,
                                     bias=half_pi[:], scale=-PI)
                # 2*sigma = th_mg + 1 ; the 0.5 is folded into cp/sp on host
                ssin = wide.tile([128, W], F16, tag="tabs", name="ssin")
                nc.vector.scalar_tensor_tensor(ssin[:], th_mg[:], 1.0, sinp[:],
                                               ALU.add, ALU.mult)
                scos = wide.tile([128, W], F16, tag="sinp", name="scos")
                nc.vector.scalar_tensor_tensor(scos[:], th_mg[:], 1.0, cosp[:],
                                               ALU.add, ALU.mult)

                # ---- key = content * pos phasor (wide, cp/sp pre-halved) ----
                cps = cpb[:, i0:i0 + SEGS, :]
                sps = spb[:, i0:i0 + SEGS, :]
                ta = wide.tile([128, W], F16, tag="tmp1", name="ta")
                nc.vector.tensor_mul(ta[:], scos[:], cps)
                tb = wide.tile([128, W], F16, tag="tmp2", name="tb")
                nc.vector.tensor_mul(tb[:], ssin[:], sps)
                kre = wide.tile([128, W], F16, tag="kre", name="kre")
                nc.vector.tensor_sub(kre[:], ta[:], tb[:])
                tc_ = wide.tile([128, W], F16, tag="tmp1", name="tc_")
                nc.vector.tensor_mul(tc_[:], ssin[:], cps)
                td = wide.tile([128, W], F16, tag="tmp2", name="td")
                nc.vector.tensor_mul(td[:], scos[:], sps)
                kim = wide.tile([128, W], F16, tag="kim", name="kim")
                nc.vector.tensor_add(kim[:], tc_[:], td[:])

                # ---- prefix scan per channel tile segment ----
                mre = wide.tile([128, W], F16, tag="mre", name="mre")
                mim = wide.tile([128, W], F16, tag="mim", name="mim")
                for s in range(SEGS):
                    seg = slice(s * c.CN, (s + 1) * c.CN)
                    init_re = 0.0 if first_in_batch else car[(h, "re")][:, s:s + 1]
                    nc.vector.tensor_tensor_scan(mre[:, seg], kre[:, seg],
                                                 kre[:, seg], init_re,
                                                 ALU.add, ALU.bypass)
                    init_im = 0.0 if first_in_batch else car[(h, "im")][:, s:s + 1]
                    nc.vector.tensor_tensor_scan(mim[:, seg], kim[:, seg],
                                                 kim[:, seg], init_im,
                                                 ALU.add, ALU.bypass)
                if (n % c.CPB) != c.CPB - 1:
                    cre = mre.rearrange("p (s t) -> p s t", s=SEGS)[:, :, c.CN - 1]
                    nc.vector.tensor_copy(car[(h, "re")][:], cre)
                    cim = mim.rearrange("p (s t) -> p s t", s=SEGS)[:, :, c.CN - 1]
                    nc.vector.tensor_copy(car[(h, "im")][:], cim)

                # ---- retrieval = state * conj(q) (wide) ----
                r1 = wide.tile([128, W], F16, tag="tmp1", name="r1")
                nc.vector.tensor_mul(r1[:], mre[:], qre[:])
                r2 = wide.tile([128, W], F16, tag="tmp2", name="r2")
                nc.vector.tensor_mul(r2[:], mim[:], qim[:])
                rre = retp.tile([128, W], F16, tag=f"ret_re_{h}",
                                name=f"ret_re_{h}")
                nc.vector.tensor_add(rre[:], r1[:], r2[:])
                r3 = wide.tile([128, W], F16, tag="tmp1", name="r3")
                nc.vector.tensor_mul(r3[:], mim[:], qre[:])
                r4 = wide.tile([128, W], F16, tag="tmp2", name="r4")
                nc.vector.tensor_mul(r4[:], mre[:], qim[:])
                rim = retp.tile([128, W], F16, tag=f"ret_im_{h}",
                                name=f"ret_im_{h}")
                nc.vector.tensor_sub(rim[:], r3[:], r4[:])
                ret_w[(h, "re")] = rre
                ret_w[(h, "im")] = rim
                # dump retrieval tiles for host-side LN stats
                # tile k index: re tiles 0..CT-1, im tiles CT..2CT-1
                nc.sync.dma_start(out=retd[:, i0:i0 + SEGS, tok], in_=rre[:])
                nc.sync.dma_start(out=retd[:, c.CT + i0:c.CT + i0 + SEGS, tok],
                                  in_=rim[:])

            # ---- proj_out partial (accumulate over all chpl tiles) ----
            ob = obp.tile([128, c.DT, c.CN], F16, tag="ob", name="ob")
            for d in range(c.DT):
                po = pout.tile([128, c.CN], F32, tag="pout")
                for k in range(c.KT2):
                    if k < c.CT:
                        h, s, pl = k // SEGS, k % SEGS, "re"
                    else:
                        h, s, pl = (k - c.CT) // SEGS, (k - c.CT) % SEGS, "im"
                    rt = ret_w[(h, pl)][:, s * c.CN:(s + 1) * c.CN]
                    nc.tensor.matmul(po[:], w2_t[k][:, d * 128:(d + 1) * 128],
                                     rt, start=(k == 0), stop=(k == c.KT2 - 1))
                nc.scalar.copy(ob[:, d, :], po[:])
            nc.sync.dma_start(out=outp[:, :, tok], in_=ob[:])

    return nc


# --------------------------------------------------------------------------
# Host-side sharding / unsharding
# --------------------------------------------------------------------------
def shard_inputs(cfg, x, W_in, W_out, ln_gamma, ln_beta, pos_phases):
    c = cfg
    HD = N_CORES * c.NCH
    xT = np.ascontiguousarray(x.reshape(c.NTOK, c.DIM).T)          # [DIM, NTOK]
    # [p, k, tok] partition-major so one DMA covers all k-tiles of a chunk
    xt_h = np.ascontiguousarray(
        xT.reshape(c.KT1, 128, c.NTOK).transpose(1, 0, 2)
    ).astype(np.float16)

    pos64 = pos_phases.astype(np.float64)
    cos_p = (0.5 * np.cos(pos64)).astype(np.float16)               # [T, HD]
    sin_p = (0.5 * np.sin(pos64)).astype(np.float16)

    Wg = (W_out * ln_gamma[None, :]).astype(np.float32)            # [DIM, 2HD]

    in_maps = []
    for cid in range(N_CORES):
        h0 = cid * c.NCH
        hs = slice(h0, h0 + c.NCH)
        w_ph = W_in[0 * HD + h0:0 * HD + h0 + c.NCH]               # [NCH, DIM]
        w_mg = W_in[1 * HD + h0:1 * HD + h0 + c.NCH]
        w_qr = W_in[2 * HD + h0:2 * HD + h0 + c.NCH]
        w_qi = W_in[3 * HD + h0:3 * HD + h0 + c.NCH]
        w_all = np.concatenate([w_ph, w_mg, w_qr, w_qi], axis=0)   # [4NCH, DIM]
        w1_h = np.ascontiguousarray(
            w_all.T.reshape(c.KT1, 128, 4 * c.NCH).transpose(1, 0, 2)
        ).astype(np.float16)

        wg_re = Wg[:, 2 * h0:2 * (h0 + c.NCH):2]                   # [DIM, NCH]
        wg_im = Wg[:, 2 * h0 + 1:2 * (h0 + c.NCH):2]
        w2T = np.concatenate([wg_re.T, wg_im.T], axis=0)           # [2NCH, DIM]
        w2_h = np.ascontiguousarray(
            w2T.reshape(c.KT2, 128, c.DIM).transpose(1, 0, 2)
        ).astype(np.float16)

        cp_h = np.ascontiguousarray(
            cos_p[:, hs].T.reshape(c.CT, 128, c.T).transpose(1, 0, 2))
        sp_h = np.ascontiguousarray(
            sin_p[:, hs].T.reshape(c.CT, 128, c.T).transpose(1, 0, 2))

        in_maps.append({
            "w1": w1_h, "w2": w2_h, "xt": xt_h,
            "cp": cp_h, "sp": sp_h,
        })
    return in_maps


def combine_outputs(cfg, results, W_out, ln_gamma, ln_beta, x_dtype):
    c = cfg
    NF = 2 * N_CORES * c.NCH
    P = np.zeros((c.DIM, c.NTOK), np.float32)
    S1 = np.zeros(c.NTOK, np.float32)
    S2 = np.zeros(c.NTOK, np.float32)
    for r in results:
        # outp is [128, DT, NTOK] partition-major of out^T -> [DIM, NTOK]
        op = r["outp"].transpose(1, 0, 2).reshape(c.DIM, c.NTOK)
        P += op.astype(np.float32)
        rd = r["retd"].astype(np.float32)          # [128, 2CT, NTOK]
        S1 += np.einsum("pkt->t", rd)
        S2 += np.einsum("pkt,pkt->t", rd, rd)
    S1 = S1.astype(np.float64)
    S2 = S2.astype(np.float64)
    mu = S1 / NF
    var = S2 / NF - mu * mu
    istd = 1.0 / np.sqrt(var + LN_EPS)
    wg_sum = (W_out.astype(np.float64) @ ln_gamma.astype(np.float64))  # [DIM]
    b_out = (W_out.astype(np.float64) @ ln_beta.astype(np.float64))    # [DIM]
    out = istd[:, None] * (P.T.astype(np.float64) - mu[:, None] * wg_sum[None, :]) \
        + b_out[None, :]
    return out.reshape(c.B, c.T, c.DIM).astype(x_dtype)


_cached = {}


def kernel(x, W_in, W_out, ln_gamma, ln_beta, pos_phases):
    cfg = Cfg(B=x.shape[0], T=x.shape[1], DIM=x.shape[2],
              NCH=pos_phases.shape[1] // N_CORES)
    key = (cfg.B, cfg.T, cfg.DIM, cfg.NCH)
    if key not in _cached:
        nc = build_program(cfg)
        split_multiwait(nc)  # walrus workaround; CoreSim path must skip this
        _cached[key] = nc
    nc = _cached[key]
    in_maps = shard_inputs(cfg, np.asarray(x), np.asarray(W_in),
                           np.asarray(W_out), np.asarray(ln_gamma),
                           np.asarray(ln_beta), np.asarray(pos_phases))
    res = run_bass_kernel_spmd(nc, in_maps, list(range(N_CORES)))
    return combine_outputs(cfg, res.results, np.asarray(W_out),
                           np.asarray(ln_gamma), np.asarray(ln_beta),
                           np.asarray(x).dtype)


# revision 6
# speedup vs baseline: 1.2867x; 1.2867x over previous
"""Trainium2 Bass kernel for nn_LongAttention (holographic long-attention block).

Computation (see reference):
  raw = x @ W_in.T -> split [c_phase | c_mag | q_re | q_im] per hd channel
  key = sigmoid(c_mag) * exp(i*(pi*tanh(c_phase) + pos_phase))
  state = cumsum_t(key);  ret = state * conj(q)
  ret_real = interleave(Re, Im) -> LayerNorm(2*hd) -> @ W_out.T

Distribution: hd (8192) split across 8 NeuronCores (1024 ch each); every core
handles both batches and all tokens. Cores are fully independent:
 - gamma is folded into W_out on the host; LayerNorm itself is algebraically
   deferred: each core returns P = ret @ (W_out*gamma).T partials plus the raw
   retrieval tiles (f16). The host computes per-token S1/S2 from the retrieval
   dump and combines:
   out = istd * (sum_c P_c - mu * (W_out @ gamma)) + W_out @ beta.
 - The cumsum runs channel-major on the DVE as a prefix scan along the free
   (time) axis, carried across token chunks -- no transposes anywhere.
 - sin/cos are evaluated via the angle-addition formula with host-precomputed
   0.5*cos/0.5*sin of pos_phases (fp16; the 0.5 cancels the sigmoid's
   (tanh+1)/2), so every ACT Sin argument is in [-pi, pi] by construction
   (the hardware LUT's valid range).

Matmuls run in fp16 (more mantissa than bf16 at the same 1 cycle/row rate),
accumulating in fp32. The elementwise chain is fp16 end-to-end: every DVE
tensor_tensor op with all-16-bit packed SBUF operands gets the 2x perf mode,
halving vector-engine busy time vs fp32.
"""

import sys
import numpy as np

for _p in ("/opt/trn_rl_repo", "/root/.axon_site/_ro/trn_rl_repo"):
    if _p not in sys.path:
        sys.path.append(_p)

import bass_rust
import concourse.bass as bass
import concourse.tile as tile
import concourse.mybir as mybir
from concourse.bass_utils import run_bass_kernel_spmd

F32 = mybir.dt.float32
F16 = mybir.dt.float16
BF16 = mybir.dt.bfloat16
AF = mybir.ActivationFunctionType
ALU = mybir.AluOpType
PI = float(np.pi)

N_CORES = 8
LN_EPS = 1e-5


# --------------------------------------------------------------------------
# Workaround: this container's walrus rejects >1 semaphore wait per
# instruction ("Too many sync wait commands"). Split the extras onto
# same-engine NoOps inserted just before (engine FIFO keeps semantics).
# --------------------------------------------------------------------------
_nop_counter = [0]


def split_multiwait(nc):
    n_split = 0
    for f in nc.m.functions:
        for bb in f.blocks:
            il = bb.instructions
            i = 0
            while i < len(il):
                ins = il[i]
                si = ins.sync_info
                waits = list(si.on_wait) if si is not None and si.on_wait else []
                if len(waits) > 1:
                    for w in waits[:-1]:
                        _nop_counter[0] += 1
                        nop = bass_rust.InstNoOp(
                            name=f"mw_nop_{_nop_counter[0]}",
                            engine=ins.engine,
                            ins=[],
                            outs=[],
                        )
                        nop.sync_info = mybir.SyncInfo(on_wait=[w], on_update=[])
                        il.insert(i, nop)
                        i += 1
                    si.on_wait = [waits[-1]]
                    n_split += 1
                i += 1
    return n_split


# --------------------------------------------------------------------------
# Device program (SPMD: identical on all cores; per-core data differs)
# --------------------------------------------------------------------------
class Cfg:
    def __init__(self, B=2, T=2048, DIM=1024, NCH=1024, CN=256):
        self.B, self.T, self.DIM, self.NCH, self.CN = B, T, DIM, NCH, CN
        self.NTOK = B * T
        self.CT = NCH // 128          # channel tiles per core
        self.KT1 = DIM // 128         # contraction tiles for proj_in
        self.KT2 = 2 * self.CT        # contraction tiles for proj_out (re+im)
        self.DT = DIM // 128          # output dim tiles
        self.NCHUNK = self.NTOK // CN
        self.CPB = T // CN            # chunks per batch


def build_program(cfg: Cfg, reps: int = 1):
    c = cfg
    assert c.CT % 4 == 0 or c.CT == 2
    SEGS = 4 if c.CT % 4 == 0 else 2   # channel tiles per wide tile
    NH = c.CT // SEGS                  # wide halves per chunk
    W = SEGS * c.CN                    # wide tile width
    nc = bass.Bass()

    w1 = nc.dram_tensor("w1", [128, c.KT1, 4 * c.NCH], F16, kind="ExternalInput")
    w2 = nc.dram_tensor("w2", [128, c.KT2, c.DIM], F16, kind="ExternalInput")
    xt = nc.dram_tensor("xt", [128, c.KT1, c.NTOK], F16, kind="ExternalInput")
    cp = nc.dram_tensor("cp", [128, c.CT, c.T], F16, kind="ExternalInput")
    sp = nc.dram_tensor("sp", [128, c.CT, c.T], F16, kind="ExternalInput")
    outp = nc.dram_tensor("outp", [128, c.DT, c.NTOK], F16, kind="ExternalOutput")
    retd = nc.dram_tensor("retd", [128, c.KT2, c.NTOK], F16, kind="ExternalOutput")

    from contextlib import ExitStack
    with tile.TileContext(nc) as tc, ExitStack() as es:
        consts = es.enter_context(tc.tile_pool(name="consts", bufs=1))
        stream = es.enter_context(tc.tile_pool(name="stream", bufs=2))
        wide = es.enter_context(tc.tile_pool(name="wide", bufs=2))
        retp = es.enter_context(tc.tile_pool(name="retp", bufs=2))
        obp = es.enter_context(tc.tile_pool(name="obp", bufs=2))
        praw = es.enter_context(tc.tile_pool(name="praw", bufs=4, space="PSUM"))
        pout = es.enter_context(tc.tile_pool(name="pout", bufs=1, space="PSUM"))

        # weights: split the load per k-tile so the first chunk's matmuls can
        # start as soon as their slice lands (instead of after the full 12 MB)
        w1_sb = consts.tile([128, c.KT1, 4 * c.NCH], F16, tag="w1_sb")
        for k in range(c.KT1):
            nc.sync.dma_start(out=w1_sb[:, k, :], in_=w1[:, k, :])
        w2_sb = consts.tile([128, c.KT2, c.DIM], F16, tag="w2_sb")
        for k in range(c.KT2):
            nc.sync.dma_start(out=w2_sb[:, k, :], in_=w2[:, k, :])
        w1_t = [w1_sb[:, k, :] for k in range(c.KT1)]
        w2_t = [w2_sb[:, k, :] for k in range(c.KT2)]

        half_pi = consts.tile([128, 1], F32, tag="half_pi")
        nc.vector.memset(half_pi[:], PI / 2)
        car = {}
        for h in range(NH):
            for pl in ("re", "im"):
                car[(h, pl)] = consts.tile([128, SEGS], F32, tag=f"car_{h}_{pl}",
                                           name=f"car_{h}_{pl}")

        for n in [nn_ for _ in range(reps) for nn_ in range(c.NCHUNK)]:
            t0 = (n % c.CPB) * c.CN
            first_in_batch = t0 == 0
            tok = slice(n * c.CN, (n + 1) * c.CN)

            xcb = stream.tile([128, c.KT1, c.CN], F16, tag="xcb")
            nc.sync.dma_start(out=xcb[:], in_=xt[:, :, tok])
            xc = [xcb[:, k, :] for k in range(c.KT1)]
            cpb = stream.tile([128, c.CT, c.CN], F16, tag="cpb")
            nc.sync.dma_start(out=cpb[:], in_=cp[:, :, t0:t0 + c.CN])
            spb = stream.tile([128, c.CT, c.CN], F16, tag="spb")
            nc.sync.dma_start(out=spb[:], in_=sp[:, :, t0:t0 + c.CN])

            ret_w = {}
            # proj_out psum accumulators: all DT tiles alive across both
            # halves so each half's contraction slice can be issued as soon
            # as that half's retrieval tiles exist (keeps PE busy during the
            # second half's elementwise chain)
            pob = [pout.tile([128, 2 * c.CN], F32, tag=f"pout{i}", name=f"po{i}")
                   for i in range(c.DT // 2)]
            po = [pob[d // 2][:, (d % 2) * c.CN:(d % 2 + 1) * c.CN]
                  for d in range(c.DT)]
            for h in range(NH):
                i0 = h * SEGS
                # ---- proj_in: 4 groups x SEGS channel tiles -> psum pairs ----
                # psum tile [128, 2*CN] holds channel tiles (j, j+1) of a group
                th_ph = wide.tile([128, W], F16, tag="th_ph", name="th_ph")
                th_mg = wide.tile([128, W], F16, tag="th_mg", name="th_mg")
                qre = wide.tile([128, W], F16, tag="qre", name="qre")
                qim = wide.tile([128, W], F16, tag="qim", name="qim")
                dest = {"ph": th_ph, "mg": th_mg, "qr": qre, "qi": qim}
                for j in range(0, SEGS, 2):
                    for gi, g in enumerate(("ph", "mg", "qr", "qi")):
                        p = praw.tile([128, 2 * c.CN], F32, tag="praw")
                        for half in range(2):
                            m = gi * c.CT + i0 + j + half
                            cols = slice(half * c.CN, (half + 1) * c.CN)
                            for k in range(c.KT1):
                                nc.tensor.matmul(
                                    p[:, cols],
                                    w1_t[k][:, m * 128:(m + 1) * 128], xc[k],
                                    start=(k == 0), stop=(k == c.KT1 - 1))
                        wcols = slice(j * c.CN, (j + 2) * c.CN)
                        if g == "ph" or g == "mg":
                            sc = 1.0 if g == "ph" else 0.5
                            nc.scalar.activation(dest[g][:, wcols], p[:],
                                                 AF.Tanh, scale=sc)
                        else:
                            nc.scalar.copy(dest[g][:, wcols], p[:])

                # ---- content phasor (wide) ----
                sinp = wide.tile([128, W], F16, tag="sinp", name="sinp")
                nc.scalar.activation(sinp[:], th_ph[:], AF.Sin, scale=PI)
                tabs = wide.tile([128, W], F16, tag="tabs", name="tabs")
                nc.scalar.activation(tabs[:], th_ph[:], AF.Abs)
                cosp = wide.tile([128, W], F16, tag="th_ph", name="cosp")
                nc.scalar.activation(cosp[:], tabs[:], AF.Sin,
                                     bias=half_pi[:], scale=-PI)
                # 2*sigma = th_mg + 1 ; the 0.5 is folded into cp/sp on host
                ssin = wide.tile([128, W], F16, tag="tabs", name="ssin")
                nc.vector.scalar_tensor_tensor(ssin[:], th_mg[:], 1.0, sinp[:],
                                               ALU.add, ALU.mult)
                scos = wide.tile([128, W], F16, tag="sinp", name="scos")
                nc.vector.scalar_tensor_tensor(scos[:], th_mg[:], 1.0, cosp[:],
                                               ALU.add, ALU.mult)

                # ---- key = content * pos phasor (wide, cp/sp pre-halved) ----
                cps = cpb[:, i0:i0 + SEGS, :]
                sps = spb[:, i0:i0 + SEGS, :]
                ta = wide.tile([128, W], F16, tag="tmp1", name="ta")
                nc.vector.tensor_mul(ta[:], scos[:], cps)
                tb = wide.tile([128, W], F16, tag="tmp2", name="tb")
                nc.vector.tensor_mul(tb[:], ssin[:], sps)
                kre = wide.tile([128, W], F16, tag="kre", name="kre")
                nc.vector.tensor_sub(kre[:], ta[:], tb[:])
                tc_ = wide.tile([128, W], F16, tag="tmp1", name="tc_")
                nc.vector.tensor_mul(tc_[:], ssin[:], cps)
                td = wide.tile([128, W], F16, tag="tmp2", name="td")
                nc.vector.tensor_mul(td[:], scos[:], sps)
                kim = wide.tile([128, W], F16, tag="kim", name="kim")
                nc.vector.tensor_add(kim[:], tc_[:], td[:])

                # ---- prefix scan per channel tile segment ----
                mre = wide.tile([128, W], F16, tag="mre", name="mre")
                mim = wide.tile([128, W], F16, tag="mim", name="mim")
                for s in range(SEGS):
                    seg = slice(s * c.CN, (s + 1) * c.CN)
                    init_re = 0.0 if first_in_batch else car[(h, "re")][:, s:s + 1]
                    nc.vector.tensor_tensor_scan(mre[:, seg], kre[:, seg],
                                                 kre[:, seg], init_re,
                                                 ALU.add, ALU.bypass)
                    init_im = 0.0 if first_in_batch else car[(h, "im")][:, s:s + 1]
                    nc.vector.tensor_tensor_scan(mim[:, seg], kim[:, seg],
                                                 kim[:, seg], init_im,
                                                 ALU.add, ALU.bypass)
                if (n % c.CPB) != c.CPB - 1:
                    cre = mre.rearrange("p (s t) -> p s t", s=SEGS)[:, :, c.CN - 1]
                    nc.vector.tensor_copy(car[(h, "re")][:], cre)
                    cim = mim.rearrange("p (s t) -> p s t", s=SEGS)[:, :, c.CN - 1]
                    nc.vector.tensor_copy(car[(h, "im")][:], cim)

                # ---- retrieval = state * conj(q) (wide) ----
                r1 = wide.tile([128, W], F16, tag="tmp1", name="r1")
                nc.vector.tensor_mul(r1[:], mre[:], qre[:])
                r2 = wide.tile([128, W], F16, tag="tmp2", name="r2")
                nc.vector.tensor_mul(r2[:], mim[:], qim[:])
                rre = retp.tile([128, W], F16, tag=f"ret_re_{h}",
                                name=f"ret_re_{h}")
                nc.vector.tensor_add(rre[:], r1[:], r2[:])
                r3 = wide.tile([128, W], F16, tag="tmp1", name="r3")
                nc.vector.tensor_mul(r3[:], mim[:], qre[:])
                r4 = wide.tile([128, W], F16, tag="tmp2", name="r4")
                nc.vector.tensor_mul(r4[:], mre[:], qim[:])
                rim = retp.tile([128, W], F16, tag=f"ret_im_{h}",
                                name=f"ret_im_{h}")
                nc.vector.tensor_sub(rim[:], r3[:], r4[:])
                ret_w[(h, "re")] = rre
                ret_w[(h, "im")] = rim
                # dump retrieval tiles for host-side LN stats
                # tile k index: re tiles 0..CT-1, im tiles CT..2CT-1
                nc.sync.dma_start(out=retd[:, i0:i0 + SEGS, tok], in_=rre[:])
                nc.sync.dma_start(out=retd[:, c.CT + i0:c.CT + i0 + SEGS, tok],
                                  in_=rim[:])

                # ---- proj_out burst for this half: contraction slices that
                # only need this half's ret tiles ----
                for d in range(c.DT):
                    for s in range(SEGS):
                        for pi, pl in enumerate(("re", "im")):
                            k = pi * c.CT + i0 + s
                            rt = ret_w[(h, pl)][:, s * c.CN:(s + 1) * c.CN]
                            nc.tensor.matmul(
                                po[d][:], w2_t[k][:, d * 128:(d + 1) * 128],
                                rt, start=(h == 0 and s == 0 and pi == 0),
                                stop=(h == NH - 1 and s == SEGS - 1 and pi == 1),
                                skip_group_check=True)

            ob = obp.tile([128, c.DT, c.CN], F16, tag="ob", name="ob")
            for d in range(c.DT):
                nc.scalar.copy(ob[:, d, :], po[d][:])
            nc.sync.dma_start(out=outp[:, :, tok], in_=ob[:])

    return nc


# --------------------------------------------------------------------------
# Host-side sharding / unsharding
# --------------------------------------------------------------------------
def shard_inputs(cfg, x, W_in, W_out, ln_gamma, ln_beta, pos_phases):
    c = cfg
    HD = N_CORES * c.NCH
    xT = np.ascontiguousarray(x.reshape(c.NTOK, c.DIM).T)          # [DIM, NTOK]
    # [p, k, tok] partition-major so one DMA covers all k-tiles of a chunk
    xt_h = np.ascontiguousarray(
        xT.reshape(c.KT1, 128, c.NTOK).transpose(1, 0, 2)
    ).astype(np.float16)

    pos64 = pos_phases.astype(np.float64)
    cos_p = (0.5 * np.cos(pos64)).astype(np.float16)               # [T, HD]
    sin_p = (0.5 * np.sin(pos64)).astype(np.float16)

    Wg = (W_out * ln_gamma[None, :]).astype(np.float32)            # [DIM, 2HD]

    in_maps = []
    for cid in range(N_CORES):
        h0 = cid * c.NCH
        hs = slice(h0, h0 + c.NCH)
        w_ph = W_in[0 * HD + h0:0 * HD + h0 + c.NCH]               # [NCH, DIM]
        w_mg = W_in[1 * HD + h0:1 * HD + h0 + c.NCH]
        w_qr = W_in[2 * HD + h0:2 * HD + h0 + c.NCH]
        w_qi = W_in[3 * HD + h0:3 * HD + h0 + c.NCH]
        w_all = np.concatenate([w_ph, w_mg, w_qr, w_qi], axis=0)   # [4NCH, DIM]
        w1_h = np.ascontiguousarray(
            w_all.T.reshape(c.KT1, 128, 4 * c.NCH).transpose(1, 0, 2)
        ).astype(np.float16)

        wg_re = Wg[:, 2 * h0:2 * (h0 + c.NCH):2]                   # [DIM, NCH]
        wg_im = Wg[:, 2 * h0 + 1:2 * (h0 + c.NCH):2]
        w2T = np.concatenate([wg_re.T, wg_im.T], axis=0)           # [2NCH, DIM]
        w2_h = np.ascontiguousarray(
            w2T.reshape(c.KT2, 128, c.DIM).transpose(1, 0, 2)
        ).astype(np.float16)

        cp_h = np.ascontiguousarray(
            cos_p[:, hs].T.reshape(c.CT, 128, c.T).transpose(1, 0, 2))
        sp_h = np.ascontiguousarray(
            sin_p[:, hs].T.reshape(c.CT, 128, c.T).transpose(1, 0, 2))

        in_maps.append({
            "w1": w1_h, "w2": w2_h, "xt": xt_h,
            "cp": cp_h, "sp": sp_h,
        })
    return in_maps


def combine_outputs(cfg, results, W_out, ln_gamma, ln_beta, x_dtype):
    c = cfg
    NF = 2 * N_CORES * c.NCH
    P = np.zeros((c.DIM, c.NTOK), np.float32)
    S1 = np.zeros(c.NTOK, np.float32)
    S2 = np.zeros(c.NTOK, np.float32)
    for r in results:
        # outp is [128, DT, NTOK] partition-major of out^T -> [DIM, NTOK]
        op = r["outp"].transpose(1, 0, 2).reshape(c.DIM, c.NTOK)
        P += op.astype(np.float32)
        rd = r["retd"].astype(np.float32)          # [128, 2CT, NTOK]
        S1 += np.einsum("pkt->t", rd)
        S2 += np.einsum("pkt,pkt->t", rd, rd)
    S1 = S1.astype(np.float64)
    S2 = S2.astype(np.float64)
    mu = S1 / NF
    var = S2 / NF - mu * mu
    istd = 1.0 / np.sqrt(var + LN_EPS)
    wg_sum = (W_out.astype(np.float64) @ ln_gamma.astype(np.float64))  # [DIM]
    b_out = (W_out.astype(np.float64) @ ln_beta.astype(np.float64))    # [DIM]
    out = istd[:, None] * (P.T.astype(np.float64) - mu[:, None] * wg_sum[None, :]) \
        + b_out[None, :]
    return out.reshape(c.B, c.T, c.DIM).astype(x_dtype)


_cached = {}


def kernel(x, W_in, W_out, ln_gamma, ln_beta, pos_phases):
    cfg = Cfg(B=x.shape[0], T=x.shape[1], DIM=x.shape[2],
              NCH=pos_phases.shape[1] // N_CORES)
    key = (cfg.B, cfg.T, cfg.DIM, cfg.NCH)
    if key not in _cached:
        nc = build_program(cfg)
        split_multiwait(nc)  # walrus workaround; CoreSim path must skip this
        _cached[key] = nc
    nc = _cached[key]
    in_maps = shard_inputs(cfg, np.asarray(x), np.asarray(W_in),
                           np.asarray(W_out), np.asarray(ln_gamma),
                           np.asarray(ln_beta), np.asarray(pos_phases))
    res = run_bass_kernel_spmd(nc, in_maps, list(range(N_CORES)))
    return combine_outputs(cfg, res.results, np.asarray(W_out),
                           np.asarray(ln_gamma), np.asarray(ln_beta),
                           np.asarray(x).dtype)
